# revision 33
# baseline (speedup 1.0000x reference)
"""Trainium2 Bass kernel: multi-head attention (B=2, S=2048, H=768, 12 heads x 64).

Sharding: 24 (batch, head) pairs over 8 cores -> 3 heads of one batch per core
(pure data/head parallel, no collectives; outputs gathered host-side).

Design (v2 — engine-balanced rewrite of the PE/ACT-bound v1):
  - Host pre-casts to bf16 AND pre-transposes hs: hs^T ([H, S]) and the
    packed QKV weight block upload directly (no PE transposes, no DVE casts,
    no XBAR — v2.0's XBAR transposes serialized ~30us on the SP queue).
    hs^T streams in 128KB chunks on the SP queue, weights on the ACT HWDGE
    queue, in parallel.
  - Weight columns packed host-side as [Q01 | K01 | K2,Q2 | V] so the two
    64-wide head-2 projections share one M=128 matmul chain.
  - V is projected NATURALLY (lhsT = hs^T tile, rhs = Wv block): no V^T
    transpose fixup. Each per-head V tile carries a ones column (M=65) so the
    ctx matmul computes the softmax denominator in PSUM row 64 for free —
    this removes v1's 192 M=1 denominator matmuls (~41us of PE time).
  - Scores per (head, kv-tile) go to single-bank PSUM tiles; exp runs
    per-bank (ACT rate is access-latency-amortized the same as 2-bank, and
    1-bank tiles free enough PSUM for double-buffered projections:
    3 score slots + 2 proj slots + 3 ctx accumulators = 8 banks).
  - Head 2's exp stream is offloaded to the idle DVE as a Schraudolph
    bit-trick: u16 = round(x*(128*log2e*0.125) + (127*128 - 7.5)) bit-viewed
    as bf16 == exp(0.125*x) within ~1.8% rms; softmax tolerates it and ACT
    drops from 127us to ~85us of exp work.
  - ctx accumulates per kv-tile immediately after that tile's exps, lagging
    scores by half a chunk; the softmax division is deferred to the host
    (ctx^T and denominators stream out as [65, 512] bf16 tiles), so the
    drain needs no PE transposes, reciprocals, or f32 output DMA.
  - bq optionally added in-kernel; bk cancels in softmax; bv added host-side.
"""

import sys

sys.path.insert(0, "/opt/trn_rl_repo")

import numpy as np
import ml_dtypes

from concourse import bacc, mybir, tile
from concourse.bass_utils import run_bass_kernel_spmd

F32 = mybir.dt.float32
BF16 = mybir.dt.bfloat16
U16 = mybir.dt.uint16
EXP = mybir.ActivationFunctionType.Exp
AOp = mybir.AluOpType

B, S, H, NH, HD = 2, 2048, 768, 12, 64
NC = 8  # cores
HPC = 3  # heads per core
DL = HPC * HD  # 192 local columns
KT = H // 128  # 6 contraction tiles
NT = S // 128  # 16 kv tiles
QC = 512  # query chunk
NQC = S // QC  # 4
MJ = 3 * DL  # 576 packed weight columns

# Schraudolph exp-as-bits constants (DVE offload of head-2 exponentials):
# u16 = round(s * SCH_A + SCH_B); u16 bits viewed as bf16 ~= exp(0.125 * s).
SCH_A = (128.0 / float(np.log(2.0))) * 0.125
SCH_B = 127.0 * 128.0 - 7.5

_CACHE = {}


def _build(use_qbias: bool):
    nc = bacc.Bacc("TRN2", target_bir_lowering=False, debug=False)
    hst_d = nc.dram_tensor("hst", [H, S], BF16, kind="ExternalInput").ap()
    wf_d = nc.dram_tensor("wf", [H * MJ], BF16, kind="ExternalInput").ap()
    out_d = nc.dram_tensor("out", [NQC, HPC, HD + 1, QC], BF16,
                           kind="ExternalOutput").ap()
    if use_qbias:
        bq_d = nc.dram_tensor("bq", [DL], F32, kind="ExternalInput").ap()

    ts = tile.bass.ts

    with tile.TileContext(nc) as tc:
        with tc.tile_pool(name="const", bufs=1) as cpool, \
             tc.tile_pool(name="qkv_sb", bufs=1) as qkv_pool, \
             tc.tile_pool(name="et_p", bufs=2) as et_pool, \
             tc.tile_pool(name="cs_p", bufs=2) as cs_pool, \
             tc.tile_pool(name="sc_ps", bufs=5, space="PSUM") as sc_pool, \
             tc.tile_pool(name="cx_ps", bufs=1, space="PSUM") as cx_pool:

            w_bb = qkv_pool.tile([128, KT, MJ], BF16)
            hsT = qkv_pool.tile([128, KT, S], BF16)
            kt01 = qkv_pool.tile([128, S], BF16)
            kt2 = qkv_pool.tile([128, S], BF16)
            qt01 = qkv_pool.tile([128, S], BF16)
            qt2 = qkv_pool.tile([128, S], BF16)
            v1 = qkv_pool.tile([128, NT, HPC, HD + 1], BF16)

            # weights on the ACT HWDGE queue, hs^T chunks on the SP queue —
            # the two streams issue in parallel during the ramp. wf is packed
            # host-side as four flat column blocks (K01 first) so one DMA
            # delivers all six k-tiles of the block the first chain needs.
            off = 0
            for lo, wd in ((128, 128), (0, 128), (256, 128), (384, DL)):
                nc.scalar.dma_start(
                    w_bb[:, :, lo : lo + wd],
                    wf_d[off : off + H * wd].rearrange(
                        "(k p c) -> p k c", k=KT, p=128, c=wd))
                off += H * wd
            for c in range(NQC):
                for k in range(KT):
                    nc.sync.dma_start(
                        hsT[:, k, ts(c, QC)], hst_d[ts(k, 128), ts(c, QC)])

            nc.vector.memset(v1[:, :, :, HD : HD + 1], 1.0)
            if use_qbias:
                bq_sb = cpool.tile([128, 2, 1], F32)
                nc.sync.dma_start(
                    bq_sb[0:128, 0, :], bq_d[0:128].rearrange("(p o) -> p o", o=1))
                nc.sync.dma_start(
                    bq_sb[0:64, 1, :], bq_d[128:192].rearrange("(p o) -> p o", o=1))

            # ---- projections (weight cols packed host-side) ----
            # wf cols: 0:128 Q01 | 128:256 K01 | 256:320 K2 + 320:384 Q2 | 384:576 V
            def k01(c):
                ps = sc_pool.tile([128, QC], F32, tag="sc", name=f"k01p{c}")
                for k in range(KT):
                    nc.tensor.matmul(ps[:], w_bb[:, k, 128:256],
                                     hsT[:, k, ts(c, QC)],
                                     start=(k == 0), stop=(k == KT - 1))
                nc.vector.tensor_copy(kt01[:, ts(c, QC)], ps[:])

            def k2q2(c):
                ps = sc_pool.tile([128, QC], F32, tag="sc", name=f"k2q2p{c}")
                for k in range(KT):
                    nc.tensor.matmul(ps[:], w_bb[:, k, 256:384],
                                     hsT[:, k, ts(c, QC)],
                                     start=(k == 0), stop=(k == KT - 1))
                # lower halves cast from PSUM on DVE; the duplicate upper
                # halves are SBUF->SBUF copies the Pool engine can own.
                nc.vector.tensor_copy(kt2[0:64, ts(c, QC)], ps[0:64, :])
                nc.gpsimd.tensor_copy(kt2[64:128, ts(c, QC)],
                                      kt2[0:64, ts(c, QC)])
                dq = qt2[0:64, ts(c, QC)]
                if use_qbias:
                    nc.vector.tensor_scalar_add(dq, ps[64:128, :],
                                                bq_sb[0:64, 1, :])
                else:
                    nc.vector.tensor_copy(dq, ps[64:128, :])
                nc.gpsimd.tensor_copy(qt2[64:128, ts(c, QC)], dq)

            def q01(c):
                ps = sc_pool.tile([128, QC], F32, tag="sc", name=f"q01p{c}")
                for k in range(KT):
                    nc.tensor.matmul(ps[:], w_bb[:, k, 0:128],
                                     hsT[:, k, ts(c, QC)],
                                     start=(k == 0), stop=(k == KT - 1))
                if use_qbias:
                    nc.vector.tensor_scalar_add(
                        qt01[:, ts(c, QC)], ps[:], bq_sb[0:128, 0, :])
                else:
                    nc.vector.tensor_copy(qt01[:, ts(c, QC)], ps[:])

            vstg = qkv_pool.tile([128, 2, DL], BF16)

            def vproj(t):
                ps = sc_pool.tile([128, DL], F32, tag="sc", name=f"vp{t}")
                for k in range(KT):
                    nc.tensor.matmul(ps[:], hsT[:, k, ts(t, 128)],
                                     w_bb[:, k, 384:576],
                                     start=(k == 0), stop=(k == KT - 1))
                # one DVE cast frees the PSUM slot fast; Pool scatters the
                # staging tile into the per-head ones-augmented layout.
                stg = vstg[:, t % 2, :]
                nc.vector.tensor_copy(stg, ps[:])
                for h in range(HPC):
                    nc.gpsimd.tensor_copy(v1[:, t, h, 0:HD],
                                          stg[:, h * HD : (h + 1) * HD])

            # ---- scores + exp ----
            ets = {}

            def alloc_et(qc):
                ets[qc] = (
                    et_pool.tile([128, NT, 2, QC], BF16, tag="et01",
                                 name=f"et01_{qc}"),
                    et_pool.tile([128, NT, QC], BF16, tag="et2",
                                 name=f"et2_{qc}"),
                )

            def _exp(eng, et_ap, ps):
                # eng: 'a' = ACT exp; 'v'/'g' = Schraudolph bits on DVE/Pool
                if eng == 'a':
                    nc.scalar.activation(et_ap, ps[:], EXP, scale=0.125)
                else:
                    e = nc.vector if eng == 'v' else nc.gpsimd
                    e.tensor_scalar(et_ap.bitcast(U16), ps[:],
                                    SCH_A, SCH_B, AOp.mult, AOp.add)

            def score_tile(qc, t, engs='aav'):
                et01, et2 = ets[qc]
                for h in range(2):
                    ps = sc_pool.tile([128, QC], F32, tag="sc",
                                      name=f"s{qc}_{t}_{h}")
                    nc.tensor.matmul(
                        ps[:],
                        kt01[h * 64 : h * 64 + 64, ts(t, 128)],
                        qt01[h * 64 : h * 64 + 64, ts(qc, QC)],
                        start=True, stop=True)
                    _exp(engs[h], et01[:, t, h, :], ps)
                hh = t % 2
                ps2 = sc_pool.tile([128, QC], F32, tag="sc",
                                   name=f"s{qc}_{t}_2")
                nc.tensor.matmul(
                    ps2[:],
                    kt2[hh * 64 : hh * 64 + 64, ts(t, 128)],
                    qt2[hh * 64 : hh * 64 + 64, ts(qc, QC)],
                    start=True, stop=True)
                _exp(engs[2], et2[:, t, :], ps2)

            # ---- ctx (+denominator via the V ones-column) ----
            cxs = {}

            def ctx_alloc(qc):
                cxs[qc] = [
                    cx_pool.tile([HD + 1, QC], F32, tag=f"cx{h}",
                                 name=f"cx{qc}_{h}")
                    for h in range(HPC)
                ]

            def ctx_partial(qc, t, start, stop):
                et01, et2 = ets[qc]
                for h in range(HPC):
                    rhs = et2[:, t, :] if h == 2 else et01[:, t, h, :]
                    nc.tensor.matmul(cxs[qc][h][:], v1[:, t, h, :], rhs,
                                     start=start, stop=stop)

            def drain(qc):
                for h in range(HPC):
                    cs = cs_pool.tile([HD + 1, QC], BF16, tag=f"cs{h}",
                                      name=f"cs{qc}_{h}")
                    nc.vector.tensor_copy(cs[:], cxs[qc][h][:])
                    nc.sync.dma_start(out_d[qc, h, :, :], cs[:])

            # ---- schedule ----
            # Uniform windows: window qc runs scores(qc) with ctx lagging two
            # kv tiles; the previous window's last two ctx partials + drain
            # land in this window's first two iterations, so the tail after
            # the final exp is just two ctx partials + drain.
            # Window 0 additionally interleaves all projections: K/Q chains
            # per chunk ahead of that chunk's score tiles, V ahead of ctx.
            alloc_et(0)
            ctx_alloc(0)
            # chunk 0's chains must precede tile 0; later chunks' chains are
            # prefetched one at a time between tiles so at most one projection
            # chain competes with the score tiles for PSUM slots.
            k01(0)
            k2q2(0)
            q01(0)
            for t in range(NT):
                c = t // 4
                if t % 4 == 1 and c < 3:
                    k01(c + 1)
                if t % 4 == 2 and c < 3:
                    k2q2(c + 1)
                if t == 2:
                    q01(1)
                vproj(t)
                score_tile(0, t)
                if t >= 2:
                    ctx_partial(0, t - 2, start=(t == 2), stop=False)

            for qc in range(1, NQC):
                alloc_et(qc)
                if qc < 3:
                    q01(qc + 1)
                for t in range(NT):
                    # Split the three exps 1.5/1.5 between ACT and DVE (the
                    # Pool engine handles the copies DVE used to own), so
                    # per-tile exp capacity ~1.05us stays under PE's ~1.25us.
                    score_tile(qc, t, 'avv' if t % 2 else 'aav')
                    if t < 2:
                        ctx_partial(qc - 1, NT - 2 + t, start=False,
                                    stop=(t == 1))
                    if t == 2:
                        drain(qc - 1)
                        ctx_alloc(qc)
                    if t >= 2:
                        ctx_partial(qc, t - 2, start=(t == 2), stop=False)
            for t in range(NT - 2, NT):
                ctx_partial(NQC - 1, t, start=False, stop=(t == NT - 1))
            drain(NQC - 1)

    nc.compile()
    return nc


def _get(use_qbias: bool):
    key = use_qbias
    if key not in _CACHE:
        _CACHE[key] = _build(use_qbias)
    return _CACHE[key]


def _make_in_maps(hidden_states, Wq, bq, Wk, Wv, use_qbias):
    in_maps = []
    for i in range(NC):
        b, g = divmod(i, NC // B)
        c0 = g * DL
        wf = np.concatenate(
            [
                np.ascontiguousarray(blk).astype(ml_dtypes.bfloat16).ravel()
                for blk in (
                    Wk[:, c0 : c0 + 128],                   # K01 (first DMA)
                    Wq[:, c0 : c0 + 128],                   # Q01
                    np.concatenate(                          # K2 | Q2
                        [Wk[:, c0 + 128 : c0 + 192],
                         Wq[:, c0 + 128 : c0 + 192]], axis=1),
                    Wv[:, c0 : c0 + DL],                    # V
                )
            ]
        )
        m = {
            "hst": np.ascontiguousarray(hidden_states[b].T).astype(
                ml_dtypes.bfloat16),
            "wf": wf,
        }
        if use_qbias:
            m["bq"] = np.ascontiguousarray(bq[c0 : c0 + DL], dtype=np.float32)
        in_maps.append(m)
    return in_maps


def _run(inputs, trace=False):
    hidden_states = np.asarray(inputs["hidden_states"], dtype=np.float32)
    Wq = np.asarray(inputs["Wq"], dtype=np.float32)
    Wk = np.asarray(inputs["Wk"], dtype=np.float32)
    Wv = np.asarray(inputs["Wv"], dtype=np.float32)
    bq = np.asarray(inputs["bq"], dtype=np.float32)
    bv = np.asarray(inputs["bv"], dtype=np.float32)
    # bk is intentionally unused: softmax over the kv axis cancels any
    # per-query constant, and q_i . bk is constant along kv.
    assert hidden_states.shape == (B, S, H)
    use_qbias = bool(np.any(bq))
    nc = _get(use_qbias)
    in_maps = _make_in_maps(hidden_states, Wq, bq, Wk, Wv, use_qbias)
    res = run_bass_kernel_spmd(nc, in_maps, core_ids=list(range(NC)), trace=trace)
    out = np.empty((B, S, H), dtype=np.float32)
    for i in range(NC):
        b, g = divmod(i, NC // B)
        c0 = g * DL
        arr = np.asarray(res.results[i]["out"]).astype(np.float32)
        ctx = arr[:, :, 0:HD, :]           # [NQC, HPC, HD, QC]
        den = arr[:, :, HD, :]             # [NQC, HPC, QC]
        blk = ctx / den[:, :, None, :]
        out[b, :, c0 : c0 + DL] = (
            blk.transpose(0, 3, 1, 2).reshape(S, DL) + bv[c0 : c0 + DL])
    return out, res


def kernel(**inputs) -> np.ndarray:
    out, _ = _run(inputs, trace=False)
    return out


# revision 35
# speedup vs baseline: 1.1573x; 1.1573x over previous
"""Trainium2 Bass kernel: multi-head attention (B=2, S=2048, H=768, 12 heads x 64).

Sharding: 24 (batch, head) pairs over 8 cores -> 3 heads of one batch per core
(pure data/head parallel, no collectives; outputs gathered host-side).

Design (v2 — engine-balanced rewrite of the PE/ACT-bound v1):
  - Host pre-casts to bf16 AND pre-transposes hs: hs^T ([H, S]) and the
    packed QKV weight block upload directly (no PE transposes, no DVE casts,
    no XBAR — v2.0's XBAR transposes serialized ~30us on the SP queue).
    hs^T streams in 128KB chunks on the SP queue, weights on the ACT HWDGE
    queue, in parallel.
  - Weight columns packed host-side as [Q01 | K01 | K2,Q2 | V] so the two
    64-wide head-2 projections share one M=128 matmul chain.
  - V is projected NATURALLY (lhsT = hs^T tile, rhs = Wv block): no V^T
    transpose fixup. Each per-head V tile carries a ones column (M=65) so the
    ctx matmul computes the softmax denominator in PSUM row 64 for free —
    this removes v1's 192 M=1 denominator matmuls (~41us of PE time).
  - Scores per (head, kv-tile) go to single-bank PSUM tiles; exp runs
    per-bank (ACT rate is access-latency-amortized the same as 2-bank, and
    1-bank tiles free enough PSUM for double-buffered projections:
    3 score slots + 2 proj slots + 3 ctx accumulators = 8 banks).
  - Head 2's exp stream is offloaded to the idle DVE as a Schraudolph
    bit-trick: u16 = round(x*(128*log2e*0.125) + (127*128 - 7.5)) bit-viewed
    as bf16 == exp(0.125*x) within ~1.8% rms; softmax tolerates it and ACT
    drops from 127us to ~85us of exp work.
  - ctx accumulates per kv-tile immediately after that tile's exps, lagging
    scores by half a chunk; the softmax division is deferred to the host
    (ctx^T and denominators stream out as [65, 512] bf16 tiles), so the
    drain needs no PE transposes, reciprocals, or f32 output DMA.
  - bq optionally added in-kernel; bk cancels in softmax; bv added host-side.
"""

import sys

sys.path.insert(0, "/opt/trn_rl_repo")

import numpy as np
import ml_dtypes

from concourse import bacc, mybir, tile
from concourse.bass_utils import run_bass_kernel_spmd

F32 = mybir.dt.float32
BF16 = mybir.dt.bfloat16
U16 = mybir.dt.uint16
EXP = mybir.ActivationFunctionType.Exp
AOp = mybir.AluOpType

B, S, H, NH, HD = 2, 2048, 768, 12, 64
NC = 8  # cores
HPC = 3  # heads per core
DL = HPC * HD  # 192 local columns
KT = H // 128  # 6 contraction tiles
NT = S // 128  # 16 kv tiles
QC = 512  # query chunk
NQC = S // QC  # 4
MJ = 3 * DL  # 576 packed weight columns

# Schraudolph exp-as-bits constants (DVE offload of head-2 exponentials):
# u16 = round(s * SCH_A + SCH_B); u16 bits viewed as bf16 ~= exp(0.125 * s).
SCH_A = (128.0 / float(np.log(2.0))) * 0.125
SCH_B = 127.0 * 128.0 - 7.5

_CACHE = {}


def _build(use_qbias: bool):
    nc = bacc.Bacc("TRN2", target_bir_lowering=False, debug=False)
    hst_d = nc.dram_tensor("hst", [H, S], BF16, kind="ExternalInput").ap()
    wf_d = nc.dram_tensor("wf", [H * MJ], BF16, kind="ExternalInput").ap()
    out_d = nc.dram_tensor("out", [NQC, HPC, HD + 1, QC], BF16,
                           kind="ExternalOutput").ap()
    if use_qbias:
        bq_d = nc.dram_tensor("bq", [DL], F32, kind="ExternalInput").ap()

    ts = tile.bass.ts

    with tile.TileContext(nc) as tc:
        with tc.tile_pool(name="const", bufs=1) as cpool, \
             tc.tile_pool(name="qkv_sb", bufs=1) as qkv_pool, \
             tc.tile_pool(name="et_p", bufs=2) as et_pool, \
             tc.tile_pool(name="cs_p", bufs=2) as cs_pool, \
             tc.tile_pool(name="sc_ps", bufs=5, space="PSUM") as sc_pool, \
             tc.tile_pool(name="cx_ps", bufs=1, space="PSUM") as cx_pool:

            w_bb = qkv_pool.tile([128, KT, MJ], BF16)
            hsT = qkv_pool.tile([128, KT, S], BF16)
            kt01 = qkv_pool.tile([128, S], BF16)
            kt2 = qkv_pool.tile([128, S], BF16)
            qt01 = qkv_pool.tile([128, S], BF16)
            qt2 = qkv_pool.tile([128, S], BF16)
            v1 = qkv_pool.tile([128, NT, HPC, HD + 1], BF16)

            # weights on the ACT HWDGE queue, hs^T chunks on the SP queue —
            # the two streams issue in parallel during the ramp. wf is packed
            # host-side as four flat column blocks (K01 first) so one DMA
            # delivers all six k-tiles of the block the first chain needs.
            off = 0
            for lo, wd in ((128, 128), (0, 128), (256, 128), (384, DL)):
                nc.scalar.dma_start(
                    w_bb[:, :, lo : lo + wd],
                    wf_d[off : off + H * wd].rearrange(
                        "(k p c) -> p k c", k=KT, p=128, c=wd))
                off += H * wd
            for c in range(NQC):
                for k in range(KT):
                    nc.sync.dma_start(
                        hsT[:, k, ts(c, QC)], hst_d[ts(k, 128), ts(c, QC)])

            nc.vector.memset(v1[:, :, :, HD : HD + 1], 1.0)
            if use_qbias:
                bq_sb = cpool.tile([128, 2, 1], F32)
                nc.sync.dma_start(
                    bq_sb[0:128, 0, :], bq_d[0:128].rearrange("(p o) -> p o", o=1))
                nc.sync.dma_start(
                    bq_sb[0:64, 1, :], bq_d[128:192].rearrange("(p o) -> p o", o=1))

            # ---- projections (weight cols packed host-side) ----
            # wf cols: 0:128 Q01 | 128:256 K01 | 256:320 K2 + 320:384 Q2 | 384:576 V
            def k01(c):
                ps = sc_pool.tile([128, QC], F32, tag="sc", name=f"k01p{c}")
                for k in range(KT):
                    nc.tensor.matmul(ps[:], w_bb[:, k, 128:256],
                                     hsT[:, k, ts(c, QC)],
                                     start=(k == 0), stop=(k == KT - 1))
                nc.vector.tensor_copy(kt01[:, ts(c, QC)], ps[:])

            def k2q2(c):
                ps = sc_pool.tile([128, QC], F32, tag="sc", name=f"k2q2p{c}")
                for k in range(KT):
                    nc.tensor.matmul(ps[:], w_bb[:, k, 256:384],
                                     hsT[:, k, ts(c, QC)],
                                     start=(k == 0), stop=(k == KT - 1))
                # lower halves cast from PSUM on DVE; the duplicate upper
                # halves are SBUF->SBUF copies the Pool engine can own.
                nc.vector.tensor_copy(kt2[0:64, ts(c, QC)], ps[0:64, :])
                nc.gpsimd.tensor_copy(kt2[64:128, ts(c, QC)],
                                      kt2[0:64, ts(c, QC)])
                dq = qt2[0:64, ts(c, QC)]
                if use_qbias:
                    nc.vector.tensor_scalar_add(dq, ps[64:128, :],
                                                bq_sb[0:64, 1, :])
                else:
                    nc.vector.tensor_copy(dq, ps[64:128, :])
                nc.gpsimd.tensor_copy(qt2[64:128, ts(c, QC)], dq)

            def q01(c):
                ps = sc_pool.tile([128, QC], F32, tag="sc", name=f"q01p{c}")
                for k in range(KT):
                    nc.tensor.matmul(ps[:], w_bb[:, k, 0:128],
                                     hsT[:, k, ts(c, QC)],
                                     start=(k == 0), stop=(k == KT - 1))
                if use_qbias:
                    nc.vector.tensor_scalar_add(
                        qt01[:, ts(c, QC)], ps[:], bq_sb[0:128, 0, :])
                else:
                    nc.vector.tensor_copy(qt01[:, ts(c, QC)], ps[:])

            def vproj(t):
                ps = sc_pool.tile([128, DL], F32, tag="sc", name=f"vp{t}")
                for k in range(KT):
                    nc.tensor.matmul(ps[:], hsT[:, k, ts(t, 128)],
                                     w_bb[:, k, 384:576],
                                     start=(k == 0), stop=(k == KT - 1))
                for h in range(HPC):
                    nc.vector.tensor_copy(v1[:, t, h, 0:HD],
                                          ps[:, h * HD : (h + 1) * HD])

            # ---- scores + exp ----
            ets = {}

            def alloc_et(qc):
                ets[qc] = (
                    et_pool.tile([128, NT, 2, QC], BF16, tag="et01",
                                 name=f"et01_{qc}"),
                    et_pool.tile([128, NT, QC], BF16, tag="et2",
                                 name=f"et2_{qc}"),
                )

            def _exp(eng, et_ap, ps):
                # eng: 'a' = ACT exp; 'v'/'g' = Schraudolph bits on DVE/Pool
                if eng == 'a':
                    nc.scalar.activation(et_ap, ps[:], EXP, scale=0.125)
                else:
                    e = nc.vector if eng == 'v' else nc.gpsimd
                    e.tensor_scalar(et_ap.bitcast(U16), ps[:],
                                    SCH_A, SCH_B, AOp.mult, AOp.add)

            def score_tile(qc, t, engs='aav'):
                et01, et2 = ets[qc]
                for h in range(2):
                    ps = sc_pool.tile([128, QC], F32, tag="sc",
                                      name=f"s{qc}_{t}_{h}")
                    nc.tensor.matmul(
                        ps[:],
                        kt01[h * 64 : h * 64 + 64, ts(t, 128)],
                        qt01[h * 64 : h * 64 + 64, ts(qc, QC)],
                        start=True, stop=True)
                    _exp(engs[h], et01[:, t, h, :], ps)
                hh = t % 2
                ps2 = sc_pool.tile([128, QC], F32, tag="sc",
                                   name=f"s{qc}_{t}_2")
                nc.tensor.matmul(
                    ps2[:],
                    kt2[hh * 64 : hh * 64 + 64, ts(t, 128)],
                    qt2[hh * 64 : hh * 64 + 64, ts(qc, QC)],
                    start=True, stop=True)
                _exp(engs[2], et2[:, t, :], ps2)

            # ---- ctx (+denominator via the V ones-column) ----
            cxs = {}

            def ctx_alloc(qc):
                cxs[qc] = [
                    cx_pool.tile([HD + 1, QC], F32, tag=f"cx{h}",
                                 name=f"cx{qc}_{h}")
                    for h in range(HPC)
                ]

            def ctx_partial(qc, t, start, stop):
                et01, et2 = ets[qc]
                for h in range(HPC):
                    rhs = et2[:, t, :] if h == 2 else et01[:, t, h, :]
                    nc.tensor.matmul(cxs[qc][h][:], v1[:, t, h, :], rhs,
                                     start=start, stop=stop)

            def drain(qc):
                for h in range(HPC):
                    cs = cs_pool.tile([HD + 1, QC], BF16, tag=f"cs{h}",
                                      name=f"cs{qc}_{h}")
                    nc.vector.tensor_copy(cs[:], cxs[qc][h][:])
                    nc.sync.dma_start(out_d[qc, h, :, :], cs[:])

            # ---- schedule ----
            # Uniform windows: window qc runs scores(qc) with ctx lagging two
            # kv tiles; the previous window's last two ctx partials + drain
            # land in this window's first two iterations, so the tail after
            # the final exp is just two ctx partials + drain.
            # Window 0 additionally interleaves all projections: K/Q chains
            # per chunk ahead of that chunk's score tiles, V ahead of ctx.
            alloc_et(0)
            ctx_alloc(0)
            for c in range(NQC):
                k01(c)
                k2q2(c)
                if c < 2:
                    q01(c)
                for t in range(4 * c, 4 * c + 4):
                    vproj(t)
                    score_tile(0, t)
                    if t >= 2:
                        ctx_partial(0, t - 2, start=(t == 2), stop=False)

            for qc in range(1, NQC):
                alloc_et(qc)
                if qc < 3:
                    q01(qc + 1)
                for t in range(NT):
                    # Split the three exps 1.5/1.5 between ACT and DVE (the
                    # Pool engine handles the copies DVE used to own), so
                    # per-tile exp capacity ~1.05us stays under PE's ~1.25us.
                    score_tile(qc, t, 'avv' if t % 2 else 'aav')
                    if t < 2:
                        ctx_partial(qc - 1, NT - 2 + t, start=False,
                                    stop=(t == 1))
                    if t == 2:
                        drain(qc - 1)
                        ctx_alloc(qc)
                    if t >= 2:
                        ctx_partial(qc, t - 2, start=(t == 2), stop=False)
            for t in range(NT - 2, NT):
                ctx_partial(NQC - 1, t, start=False, stop=(t == NT - 1))
            drain(NQC - 1)

    nc.compile()
    return nc


def _get(use_qbias: bool):
    key = use_qbias
    if key not in _CACHE:
        _CACHE[key] = _build(use_qbias)
    return _CACHE[key]


def _make_in_maps(hidden_states, Wq, bq, Wk, Wv, use_qbias):
    in_maps = []
    for i in range(NC):
        b, g = divmod(i, NC // B)
        c0 = g * DL
        wf = np.concatenate(
            [
                np.ascontiguousarray(blk).astype(ml_dtypes.bfloat16).ravel()
                for blk in (
                    Wk[:, c0 : c0 + 128],                   # K01 (first DMA)
                    Wq[:, c0 : c0 + 128],                   # Q01
                    np.concatenate(                          # K2 | Q2
                        [Wk[:, c0 + 128 : c0 + 192],
                         Wq[:, c0 + 128 : c0 + 192]], axis=1),
                    Wv[:, c0 : c0 + DL],                    # V
                )
            ]
        )
        m = {
            "hst": np.ascontiguousarray(hidden_states[b].T).astype(
                ml_dtypes.bfloat16),
            "wf": wf,
        }
        if use_qbias:
            m["bq"] = np.ascontiguousarray(bq[c0 : c0 + DL], dtype=np.float32)
        in_maps.append(m)
    return in_maps


def _run(inputs, trace=False):
    hidden_states = np.asarray(inputs["hidden_states"], dtype=np.float32)
    Wq = np.asarray(inputs["Wq"], dtype=np.float32)
    Wk = np.asarray(inputs["Wk"], dtype=np.float32)
    Wv = np.asarray(inputs["Wv"], dtype=np.float32)
    bq = np.asarray(inputs["bq"], dtype=np.float32)
    bv = np.asarray(inputs["bv"], dtype=np.float32)
    # bk is intentionally unused: softmax over the kv axis cancels any
    # per-query constant, and q_i . bk is constant along kv.
    assert hidden_states.shape == (B, S, H)
    use_qbias = bool(np.any(bq))
    nc = _get(use_qbias)
    in_maps = _make_in_maps(hidden_states, Wq, bq, Wk, Wv, use_qbias)
    res = run_bass_kernel_spmd(nc, in_maps, core_ids=list(range(NC)), trace=trace)
    out = np.empty((B, S, H), dtype=np.float32)
    for i in range(NC):
        b, g = divmod(i, NC // B)
        c0 = g * DL
        arr = np.asarray(res.results[i]["out"]).astype(np.float32)
        ctx = arr[:, :, 0:HD, :]           # [NQC, HPC, HD, QC]
        den = arr[:, :, HD, :]             # [NQC, HPC, QC]
        blk = ctx / den[:, :, None, :]
        out[b, :, c0 : c0 + DL] = (
            blk.transpose(0, 3, 1, 2).reshape(S, DL) + bv[c0 : c0 + DL])
    return out, res


def kernel(**inputs) -> np.ndarray:
    out, _ = _run(inputs, trace=False)
    return out


# revision 36
# speedup vs baseline: 1.1663x; 1.0078x over previous
"""Trainium2 Bass kernel: multi-head attention (B=2, S=2048, H=768, 12 heads x 64).

Sharding: 24 (batch, head) pairs over 8 cores -> 3 heads of one batch per core
(pure data/head parallel, no collectives; outputs gathered host-side).

Design (v2 — engine-balanced rewrite of the PE/ACT-bound v1):
  - Host pre-casts to bf16 AND pre-transposes hs: hs^T ([H, S]) and the
    packed QKV weight block upload directly (no PE transposes, no DVE casts,
    no XBAR — v2.0's XBAR transposes serialized ~30us on the SP queue).
    hs^T streams in 128KB chunks on the SP queue, weights on the ACT HWDGE
    queue, in parallel.
  - Weight columns packed host-side as [Q01 | K01 | K2,Q2 | V] so the two
    64-wide head-2 projections share one M=128 matmul chain.
  - V is projected NATURALLY (lhsT = hs^T tile, rhs = Wv block): no V^T
    transpose fixup. Each per-head V tile carries a ones column (M=65) so the
    ctx matmul computes the softmax denominator in PSUM row 64 for free —
    this removes v1's 192 M=1 denominator matmuls (~41us of PE time).
  - Scores per (head, kv-tile) go to single-bank PSUM tiles; exp runs
    per-bank (ACT rate is access-latency-amortized the same as 2-bank, and
    1-bank tiles free enough PSUM for double-buffered projections:
    3 score slots + 2 proj slots + 3 ctx accumulators = 8 banks).
  - Head 2's exp stream is offloaded to the idle DVE as a Schraudolph
    bit-trick: u16 = round(x*(128*log2e*0.125) + (127*128 - 7.5)) bit-viewed
    as bf16 == exp(0.125*x) within ~1.8% rms; softmax tolerates it and ACT
    drops from 127us to ~85us of exp work.
  - ctx accumulates per kv-tile immediately after that tile's exps, lagging
    scores by half a chunk; the softmax division is deferred to the host
    (ctx^T and denominators stream out as [65, 512] bf16 tiles), so the
    drain needs no PE transposes, reciprocals, or f32 output DMA.
  - bq optionally added in-kernel; bk cancels in softmax; bv added host-side.
"""

import sys

sys.path.insert(0, "/opt/trn_rl_repo")

import numpy as np
import ml_dtypes

from concourse import bacc, mybir, tile
from concourse.bass_utils import run_bass_kernel_spmd

F32 = mybir.dt.float32
BF16 = mybir.dt.bfloat16
U16 = mybir.dt.uint16
EXP = mybir.ActivationFunctionType.Exp
AOp = mybir.AluOpType

B, S, H, NH, HD = 2, 2048, 768, 12, 64
NC = 8  # cores
HPC = 3  # heads per core
DL = HPC * HD  # 192 local columns
KT = H // 128  # 6 contraction tiles
NT = S // 128  # 16 kv tiles
QC = 512  # query chunk
NQC = S // QC  # 4
MJ = 3 * DL  # 576 packed weight columns

# Schraudolph exp-as-bits constants (DVE offload of head-2 exponentials):
# u16 = round(s * SCH_A + SCH_B); u16 bits viewed as bf16 ~= exp(0.125 * s).
SCH_A = (128.0 / float(np.log(2.0))) * 0.125
SCH_B = 127.0 * 128.0 - 7.5

_CACHE = {}


def _build(use_qbias: bool):
    nc = bacc.Bacc("TRN2", target_bir_lowering=False, debug=False)
    hst_d = nc.dram_tensor("hst", [H, S], BF16, kind="ExternalInput").ap()
    wf_d = nc.dram_tensor("wf", [H * MJ], BF16, kind="ExternalInput").ap()
    out_d = nc.dram_tensor("out", [NQC, HPC, HD + 1, QC], BF16,
                           kind="ExternalOutput").ap()
    if use_qbias:
        bq_d = nc.dram_tensor("bq", [DL], F32, kind="ExternalInput").ap()

    ts = tile.bass.ts

    with tile.TileContext(nc) as tc:
        with tc.tile_pool(name="const", bufs=1) as cpool, \
             tc.tile_pool(name="qkv_sb", bufs=1) as qkv_pool, \
             tc.tile_pool(name="et_p", bufs=2) as et_pool, \
             tc.tile_pool(name="cs_p", bufs=2) as cs_pool, \
             tc.tile_pool(name="sc_ps", bufs=5, space="PSUM") as sc_pool, \
             tc.tile_pool(name="cx_ps", bufs=1, space="PSUM") as cx_pool:

            w_bb = qkv_pool.tile([128, KT, MJ], BF16)
            hsT = qkv_pool.tile([128, KT, S], BF16)
            kt01 = qkv_pool.tile([128, S], BF16)
            kt2 = qkv_pool.tile([128, S], BF16)
            qt01 = qkv_pool.tile([128, S], BF16)
            qt2 = qkv_pool.tile([128, S], BF16)
            v1 = qkv_pool.tile([128, NT, HPC, HD + 1], BF16)

            # weights on the ACT HWDGE queue, hs^T chunks on the SP queue —
            # the two streams issue in parallel during the ramp. wf is packed
            # host-side as four flat column blocks (K01 first) so one DMA
            # delivers all six k-tiles of the block the first chain needs.
            off = 0
            for lo, wd in ((128, 128), (0, 128), (256, 128), (384, DL)):
                nc.scalar.dma_start(
                    w_bb[:, :, lo : lo + wd],
                    wf_d[off : off + H * wd].rearrange(
                        "(k p c) -> p k c", k=KT, p=128, c=wd))
                off += H * wd
            for c in range(NQC):
                for k in range(KT):
                    nc.sync.dma_start(
                        hsT[:, k, ts(c, QC)], hst_d[ts(k, 128), ts(c, QC)])

            nc.vector.memset(v1[:, :, :, HD : HD + 1], 1.0)
            if use_qbias:
                bq_sb = cpool.tile([128, 2, 1], F32)
                nc.sync.dma_start(
                    bq_sb[0:128, 0, :], bq_d[0:128].rearrange("(p o) -> p o", o=1))
                nc.sync.dma_start(
                    bq_sb[0:64, 1, :], bq_d[128:192].rearrange("(p o) -> p o", o=1))

            # ---- projections (weight cols packed host-side) ----
            # wf cols: 0:128 Q01 | 128:256 K01 | 256:320 K2 + 320:384 Q2 | 384:576 V
            def k01(c):
                ps = sc_pool.tile([128, QC], F32, tag="sc", name=f"k01p{c}")
                for k in range(KT):
                    nc.tensor.matmul(ps[:], w_bb[:, k, 128:256],
                                     hsT[:, k, ts(c, QC)],
                                     start=(k == 0), stop=(k == KT - 1))
                nc.vector.tensor_copy(kt01[:, ts(c, QC)], ps[:])

            def k2q2(c):
                ps = sc_pool.tile([128, QC], F32, tag="sc", name=f"k2q2p{c}")
                for k in range(KT):
                    nc.tensor.matmul(ps[:], w_bb[:, k, 256:384],
                                     hsT[:, k, ts(c, QC)],
                                     start=(k == 0), stop=(k == KT - 1))
                # lower halves cast from PSUM on DVE; the duplicate upper
                # halves are SBUF->SBUF copies the Pool engine can own.
                nc.vector.tensor_copy(kt2[0:64, ts(c, QC)], ps[0:64, :])
                nc.gpsimd.tensor_copy(kt2[64:128, ts(c, QC)],
                                      kt2[0:64, ts(c, QC)])
                dq = qt2[0:64, ts(c, QC)]
                if use_qbias:
                    nc.vector.tensor_scalar_add(dq, ps[64:128, :],
                                                bq_sb[0:64, 1, :])
                else:
                    nc.vector.tensor_copy(dq, ps[64:128, :])
                nc.gpsimd.tensor_copy(qt2[64:128, ts(c, QC)], dq)

            def q01(c):
                ps = sc_pool.tile([128, QC], F32, tag="sc", name=f"q01p{c}")
                for k in range(KT):
                    nc.tensor.matmul(ps[:], w_bb[:, k, 0:128],
                                     hsT[:, k, ts(c, QC)],
                                     start=(k == 0), stop=(k == KT - 1))
                if use_qbias:
                    nc.vector.tensor_scalar_add(
                        qt01[:, ts(c, QC)], ps[:], bq_sb[0:128, 0, :])
                else:
                    nc.vector.tensor_copy(qt01[:, ts(c, QC)], ps[:])

            def vproj(t):
                ps = sc_pool.tile([128, DL], F32, tag="sc", name=f"vp{t}")
                for k in range(KT):
                    nc.tensor.matmul(ps[:], hsT[:, k, ts(t, 128)],
                                     w_bb[:, k, 384:576],
                                     start=(k == 0), stop=(k == KT - 1))
                for h in range(HPC):
                    nc.vector.tensor_copy(v1[:, t, h, 0:HD],
                                          ps[:, h * HD : (h + 1) * HD])

            # ---- scores + exp ----
            ets = {}

            def alloc_et(qc):
                ets[qc] = (
                    et_pool.tile([128, NT, 2, QC], BF16, tag="et01",
                                 name=f"et01_{qc}"),
                    et_pool.tile([128, NT, QC], BF16, tag="et2",
                                 name=f"et2_{qc}"),
                )

            def _exp(eng, et_ap, ps):
                # eng: 'a' = ACT exp; 'v'/'g' = Schraudolph bits on DVE/Pool
                if eng == 'a':
                    nc.scalar.activation(et_ap, ps[:], EXP, scale=0.125)
                else:
                    e = nc.vector if eng == 'v' else nc.gpsimd
                    e.tensor_scalar(et_ap.bitcast(U16), ps[:],
                                    SCH_A, SCH_B, AOp.mult, AOp.add)

            def score_tile(qc, t, engs='aav'):
                et01, et2 = ets[qc]
                for h in range(2):
                    ps = sc_pool.tile([128, QC], F32, tag="sc",
                                      name=f"s{qc}_{t}_{h}")
                    nc.tensor.matmul(
                        ps[:],
                        kt01[h * 64 : h * 64 + 64, ts(t, 128)],
                        qt01[h * 64 : h * 64 + 64, ts(qc, QC)],
                        start=True, stop=True)
                    _exp(engs[h], et01[:, t, h, :], ps)
                hh = t % 2
                ps2 = sc_pool.tile([128, QC], F32, tag="sc",
                                   name=f"s{qc}_{t}_2")
                nc.tensor.matmul(
                    ps2[:],
                    kt2[hh * 64 : hh * 64 + 64, ts(t, 128)],
                    qt2[hh * 64 : hh * 64 + 64, ts(qc, QC)],
                    start=True, stop=True)
                _exp(engs[2], et2[:, t, :], ps2)

            # ---- ctx (+denominator via the V ones-column) ----
            cxs = {}

            def ctx_alloc(qc):
                cxs[qc] = [
                    cx_pool.tile([HD + 1, QC], F32, tag=f"cx{h}",
                                 name=f"cx{qc}_{h}")
                    for h in range(HPC)
                ]

            def ctx_partial(qc, t, start, stop):
                et01, et2 = ets[qc]
                for h in range(HPC):
                    rhs = et2[:, t, :] if h == 2 else et01[:, t, h, :]
                    nc.tensor.matmul(cxs[qc][h][:], v1[:, t, h, :], rhs,
                                     start=start, stop=stop)

            def drain(qc):
                for h in range(HPC):
                    cs = cs_pool.tile([HD + 1, QC], BF16, tag=f"cs{h}",
                                      name=f"cs{qc}_{h}")
                    nc.vector.tensor_copy(cs[:], cxs[qc][h][:])
                    nc.sync.dma_start(out_d[qc, h, :, :], cs[:])

            # ---- schedule ----
            # Uniform windows: window qc runs scores(qc) with ctx lagging two
            # kv tiles; the previous window's last two ctx partials + drain
            # land in this window's first two iterations, so the tail after
            # the final exp is just two ctx partials + drain.
            # Window 0 additionally interleaves all projections: K/Q chains
            # per chunk ahead of that chunk's score tiles, V ahead of ctx.
            alloc_et(0)
            ctx_alloc(0)
            for c in range(NQC):
                k01(c)
                k2q2(c)
                if c < 2:
                    q01(c)
                for t in range(4 * c, 4 * c + 4):
                    vproj(t)
                    # window 0's DVE also carries the V copies and projection
                    # casts — keep it under PE's pace by giving ACT half the
                    # head-2 exps here.
                    score_tile(0, t, 'aav' if t % 2 else 'aaa')
                    if t >= 2:
                        ctx_partial(0, t - 2, start=(t == 2), stop=False)

            for qc in range(1, NQC):
                alloc_et(qc)
                if qc < 3:
                    q01(qc + 1)
                for t in range(NT):
                    # Split the three exps 1.5/1.5 between ACT and DVE (the
                    # Pool engine handles the copies DVE used to own), so
                    # per-tile exp capacity ~1.05us stays under PE's ~1.25us.
                    score_tile(qc, t, 'avv' if t % 2 else 'aav')
                    if t < 2:
                        ctx_partial(qc - 1, NT - 2 + t, start=False,
                                    stop=(t == 1))
                    if t == 2:
                        drain(qc - 1)
                        ctx_alloc(qc)
                    if t >= 2:
                        ctx_partial(qc, t - 2, start=(t == 2), stop=False)
            for t in range(NT - 2, NT):
                ctx_partial(NQC - 1, t, start=False, stop=(t == NT - 1))
            drain(NQC - 1)

    nc.compile()
    return nc


def _get(use_qbias: bool):
    key = use_qbias
    if key not in _CACHE:
        _CACHE[key] = _build(use_qbias)
    return _CACHE[key]


def _make_in_maps(hidden_states, Wq, bq, Wk, Wv, use_qbias):
    in_maps = []
    for i in range(NC):
        b, g = divmod(i, NC // B)
        c0 = g * DL
        wf = np.concatenate(
            [
                np.ascontiguousarray(blk).astype(ml_dtypes.bfloat16).ravel()
                for blk in (
                    Wk[:, c0 : c0 + 128],                   # K01 (first DMA)
                    Wq[:, c0 : c0 + 128],                   # Q01
                    np.concatenate(                          # K2 | Q2
                        [Wk[:, c0 + 128 : c0 + 192],
                         Wq[:, c0 + 128 : c0 + 192]], axis=1),
                    Wv[:, c0 : c0 + DL],                    # V
                )
            ]
        )
        m = {
            "hst": np.ascontiguousarray(hidden_states[b].T).astype(
                ml_dtypes.bfloat16),
            "wf": wf,
        }
        if use_qbias:
            m["bq"] = np.ascontiguousarray(bq[c0 : c0 + DL], dtype=np.float32)
        in_maps.append(m)
    return in_maps


def _run(inputs, trace=False):
    hidden_states = np.asarray(inputs["hidden_states"], dtype=np.float32)
    Wq = np.asarray(inputs["Wq"], dtype=np.float32)
    Wk = np.asarray(inputs["Wk"], dtype=np.float32)
    Wv = np.asarray(inputs["Wv"], dtype=np.float32)
    bq = np.asarray(inputs["bq"], dtype=np.float32)
    bv = np.asarray(inputs["bv"], dtype=np.float32)
    # bk is intentionally unused: softmax over the kv axis cancels any
    # per-query constant, and q_i . bk is constant along kv.
    assert hidden_states.shape == (B, S, H)
    use_qbias = bool(np.any(bq))
    nc = _get(use_qbias)
    in_maps = _make_in_maps(hidden_states, Wq, bq, Wk, Wv, use_qbias)
    res = run_bass_kernel_spmd(nc, in_maps, core_ids=list(range(NC)), trace=trace)
    out = np.empty((B, S, H), dtype=np.float32)
    for i in range(NC):
        b, g = divmod(i, NC // B)
        c0 = g * DL
        arr = np.asarray(res.results[i]["out"]).astype(np.float32)
        ctx = arr[:, :, 0:HD, :]           # [NQC, HPC, HD, QC]
        den = arr[:, :, HD, :]             # [NQC, HPC, QC]
        blk = ctx / den[:, :, None, :]
        out[b, :, c0 : c0 + DL] = (
            blk.transpose(0, 3, 1, 2).reshape(S, DL) + bv[c0 : c0 + DL])
    return out, res


def kernel(**inputs) -> np.ndarray:
    out, _ = _run(inputs, trace=False)
    return out
